# revision 2
# baseline (speedup 1.0000x reference)
"""GNN message-passing encoder (GatedGraphConv-style) on 8 Trainium2 NeuronCores.

Strategy (dst-sharded, gather-only, scatter-free):
  - Nodes are partitioned across 8 cores (12500 rows each); each core owns the
    edges whose dst falls in its range.
  - Per layer every core computes m = h @ W for its slice; the table is
    distributed via 4 per-quarter AllGathers (chunk q = quarter q of every
    core, <=25000 rows so gpsimd.dma_gather's int16 indices reach it). The
    collectives fire while the previous layer's GRU wave is still running.
  - The per-core edge stream is laid out host-side as window-group-major
    blocks: groups of WG=4 dst-windows (128 nodes each), within a group the 4
    src-chunks back to back, every (chunk, window) block padded to a uniform B
    tiles of 128 slots, so the SPMD program is identical on all cores.
  - The scatter-add (segment-sum over dst) is replaced by TensorE matmuls with
    data-built one-hot selection matrices (is_equal of a per-slot dst-column
    stream against an iota row). All 4*B tiles of one window accumulate into
    one PSUM bank; ScalarE copies the finished window into SBUF. Pad slots
    carry a sentinel column so they match nothing.
  - The GRU cell (PE matmuls + ACT sigmoid/tanh + DVE elementwise) for a
    window group runs while the next group is being gathered; the next layer's
    m tile is produced right after each GRU tile. Graph pooling uses the same
    onehot-matmul trick over the sorted batch vector; each core emits a
    [256, 64] partial pooled sum and the host adds the 8 partials.
"""

import sys

for _p in ("/opt/trn_rl_repo", "/root/.axon_site/_ro/trn_rl_repo"):
    if _p not in sys.path:
        sys.path.insert(0, _p)

import numpy as np
import ml_dtypes

P = 128
N_CORES = 8
N_CHUNKS = 4
WG = 4                 # windows per group
PAD_SENTINEL = 999.0

_cache = {}


def _wrap16(idx, channels=128):
    n = len(idx)
    a = np.asarray(idx, np.int16).reshape(n // 16, 16).T
    return np.ascontiguousarray(np.tile(a, (channels // 16, 1)))


def _host_prep(x, edge_index, batch):
    N, D = x.shape
    NPC = N // N_CORES
    QS = (NPC + N_CHUNKS - 1) // N_CHUNKS
    qsizes = [min(QS, NPC - q * QS) for q in range(N_CHUNKS)]
    NW = (NPC + P - 1) // P
    NWG = (NW + WG - 1) // WG

    src = np.asarray(edge_index[0], np.int64)
    dst = np.asarray(edge_index[1], np.int64)
    batch = np.asarray(batch, np.int64)

    # quarter-relabeled table position: chunk q holds every core's q-th quarter
    src_core = src // NPC
    src_l = src % NPC
    src_chunk = np.minimum(src_l // QS, N_CHUNKS - 1)
    src_local = src_core * np.array(qsizes)[src_chunk] + (src_l - src_chunk * QS)

    # stream block order: (window_group, chunk, window_in_group)
    def block_id(c, w):
        return (w // WG) * (N_CHUNKS * WG) + c * WG + (w % WG)

    dst_core = dst // NPC
    per_core = []
    B = 1
    n_blocks = NWG * N_CHUNKS * WG  # includes ghost blocks of a short last group
    for k in range(N_CORES):
        sel = dst_core == k
        s_loc = src_local[sel]
        c = src_chunk[sel]
        dl = dst[sel] - k * NPC
        w = dl // P
        key = block_id(c, w)
        order = np.argsort(key, kind="stable")
        s_loc, dl, w, key = s_loc[order], dl[order], w[order], key[order]
        cnt = np.bincount(key, minlength=n_blocks)
        B = max(B, int(-(-cnt.max() // P)))
        per_core.append((s_loc, dl, w, key, cnt))

    S = n_blocks * B * P  # padded slots per core (ghost blocks included)
    cores = []
    for k in range(N_CORES):
        s_loc, dl, w, key, cnt = per_core[k]
        starts = np.zeros(n_blocks, np.int64)
        starts[1:] = np.cumsum(cnt)[:-1]
        rank = np.arange(len(s_loc)) - starts[key]
        pos = key * (B * P) + rank
        gidx = np.zeros(S, np.int16)
        gidx[pos] = s_loc.astype(np.int16)
        dcol = np.full(S, PAD_SENTINEL, np.float32)
        dcol[pos] = (dl - w * P).astype(np.float32)
        bslice = batch[k * NPC:(k + 1) * NPC]
        bc0 = np.full(NW * P, PAD_SENTINEL, np.float32)
        bc1 = np.full(NW * P, PAD_SENTINEL, np.float32)
        bc0[:NPC] = np.where(bslice < 128, bslice, PAD_SENTINEL)
        bc1[:NPC] = np.where(bslice >= 128, bslice - 128, PAD_SENTINEL)
        cores.append(dict(
            gidx=_wrap16(gidx),
            dcol=np.ascontiguousarray(
                dcol.reshape(S // P, P).T.astype(ml_dtypes.bfloat16)),
            bc0=np.ascontiguousarray(bc0.reshape(NW, P).T),
            bc1=np.ascontiguousarray(bc1.reshape(NW, P).T),
        ))
    return dict(N=N, D=D, NPC=NPC, QS=QS, qsizes=qsizes, NW=NW, NWG=NWG, B=B,
                S=S, cores=cores)


def _build_program(meta, n_layers):
    import concourse.bacc as bacc
    import concourse.mybir as mybir
    import concourse.tile as tile
    from concourse.library_config import mlp as mlp_lib

    N, D, NPC, QS, NW, NWG, B, S = (meta[z] for z in
                                    ("N", "D", "NPC", "QS", "NW", "NWG", "B", "S"))
    qsizes = meta["qsizes"]
    NT_LAST = NPC - (NW - 1) * P
    f32 = mybir.dt.float32
    bf16 = mybir.dt.bfloat16
    AF = mybir.ActivationFunctionType

    nc = bacc.Bacc("TRN2", target_bir_lowering=False, debug=False,
                   num_swdge_queues=4)

    xs = nc.dram_tensor("xs", [NPC, D], f32, kind="ExternalInput")
    gidx = nc.dram_tensor("gidx", [128, S // 16], mybir.dt.int16, kind="ExternalInput")
    dcol = nc.dram_tensor("dcol", [128, S // P], bf16, kind="ExternalInput")
    bc0 = nc.dram_tensor("bc0", [128, NW], f32, kind="ExternalInput")
    bc1 = nc.dram_tensor("bc1", [128, NW], f32, kind="ExternalInput")
    iota4 = nc.dram_tensor("iota4", [128, 4 * P], bf16, kind="ExternalInput")
    iotaf = nc.dram_tensor("iotaf", [128, P], f32, kind="ExternalInput")
    ident = nc.dram_tensor("ident", [128, P], f32, kind="ExternalInput")
    ones1 = nc.dram_tensor("ones1", [1, P], f32, kind="ExternalInput")
    wmat = nc.dram_tensor("wmat", [D, n_layers * D], f32, kind="ExternalInput")
    wihT = nc.dram_tensor("wihT", [D, 3 * D], f32, kind="ExternalInput")
    whhT = nc.dram_tensor("whhT", [D, 3 * D], f32, kind="ExternalInput")
    biasA = nc.dram_tensor("biasA", [1, 3 * D], f32, kind="ExternalInput")
    biasB = nc.dram_tensor("biasB", [1, D], f32, kind="ExternalInput")
    pooled = nc.dram_tensor("pooled", [256, D], f32, kind="ExternalOutput")

    with tile.TileContext(nc) as tc:
        with (
            tc.tile_pool(name="const", bufs=1) as cpool,
            tc.tile_pool(name="state", bufs=1) as spool,
            tc.tile_pool(name="gbuf", bufs=14) as gpool,
            tc.tile_pool(name="oh", bufs=8) as opool,
            tc.tile_pool(name="aggw", bufs=8) as apool,
            tc.tile_pool(name="tmp", bufs=8) as tpool,
            tc.tile_pool(name="red", bufs=4, space="PSUM") as rpool,
            tc.tile_pool(name="gru", bufs=4, space="PSUM") as upool,
            tc.tile_pool(name="dram", bufs=1, space="DRAM") as dpool,
        ):
            gidx_sb = spool.tile([128, S // 16], mybir.dt.int16, tag="gidx")
            dcol_sb = spool.tile([128, S // P], bf16, tag="dcol")
            h_row = spool.tile([128, NW, D], f32, tag="hrow")
            hT = spool.tile([64, NW * P], f32, tag="hT")
            iota4_sb = cpool.tile([128, 4 * P], bf16, tag="iota4")
            iotaf_sb = cpool.tile([128, P], f32, tag="iotaf")
            ident_sb = cpool.tile([128, P], f32, tag="ident")
            ones1_sb = cpool.tile([1, P], f32, tag="ones1")
            wmat_sb = cpool.tile([D, n_layers * D], f32, tag="wmat")
            wihT_sb = cpool.tile([D, 3 * D], f32, tag="wihT")
            whhT_sb = cpool.tile([D, 3 * D], f32, tag="whhT")
            biasA_sb = cpool.tile([1, 3 * D], f32, tag="biasA")
            biasB_sb = cpool.tile([1, D], f32, tag="biasB")
            bc0_sb = cpool.tile([128, NW], f32, tag="bc0")
            bc1_sb = cpool.tile([128, NW], f32, tag="bc1")

            nc.sync.dma_start(gidx_sb[:], gidx[:])
            nc.sync.dma_start(dcol_sb[:], dcol[:])
            nc.sync.dma_start(iota4_sb[:], iota4[:])
            nc.sync.dma_start(iotaf_sb[:], iotaf[:])
            nc.sync.dma_start(ident_sb[:], ident[:])
            nc.sync.dma_start(ones1_sb[:], ones1[:])
            nc.sync.dma_start(wmat_sb[:], wmat[:])
            nc.sync.dma_start(wihT_sb[:], wihT[:])
            nc.sync.dma_start(whhT_sb[:], whhT[:])
            nc.sync.dma_start(biasA_sb[:], biasA[:])
            nc.sync.dma_start(biasB_sb[:], biasB[:])
            nc.sync.dma_start(bc0_sb[:], bc0[:])
            nc.sync.dma_start(bc1_sb[:], bc1[:])

            qstart = [sum(qsizes[:q]) for q in range(N_CHUNKS)]
            m_bounces = []
            m_chunks = []
            for layer in range(n_layers):
                mb = dpool.tile([NPC, 2 * D], bf16, tag=f"mb{layer}", name=f"mb{layer}")
                chs = []
                for q in range(N_CHUNKS):
                    ch = dpool.tile([N_CORES * qsizes[q], 2 * D], bf16,
                                    addr_space="Shared", tag=f"mf{layer}_{q}",
                                    name=f"mf{layer}_{q}")
                    chs.append(ch)
                m_bounces.append(mb)
                m_chunks.append(chs)

            def emit_m_tile(layer, t, rows):
                psm = upool.tile([128, D], f32, tag="gru", name=f"psm{layer}_{t}")
                nc.tensor.matmul(psm[:], lhsT=hT[:, t * P:(t + 1) * P],
                                 rhs=wmat_sb[:, layer * D:(layer + 1) * D],
                                 start=True, stop=True)
                mt = tpool.tile([128, 2 * D], bf16, tag="mt", name=f"mt{layer}_{t}")
                nc.scalar.activation(mt[:, 0:D], psm[:], AF.Copy)
                nc.scalar.activation(mt[:, D:2 * D], psm[:], AF.Copy)
                nc.sync.dma_start(
                    m_bounces[layer][t * P:t * P + rows, :], mt[:rows, :])

            def emit_collectives(layer):
                for q in range(N_CHUNKS):
                    nc.gpsimd.collective_compute(
                        "AllGather", mybir.AluOpType.bypass,
                        ins=[m_bounces[layer][qstart[q]:qstart[q] + qsizes[q], :]],
                        outs=[m_chunks[layer][q][:]],
                        replica_groups=[list(range(N_CORES))],
                    )

            # ---- init: load x -> h_row, build hT, layer-0 m + collectives ----
            nc.gpsimd.memset(h_row[:], 0.0)
            nfull = NPC // P
            nc.sync.dma_start(
                h_row[:, :nfull, :],
                xs[:nfull * P, :].rearrange("(t p) d -> p t d", p=P),
            )
            if NT_LAST < P:
                nc.sync.dma_start(h_row[:NT_LAST, nfull, :], xs[nfull * P:, :])
            for t in range(NW):
                pst = upool.tile([64, P], f32, tag="gru", name=f"pst_i{t}")
                nc.tensor.transpose(pst[:], h_row[:, t, :], ident_sb[:])
                nc.scalar.activation(hT[:, t * P:(t + 1) * P], pst[:], AF.Copy)
                emit_m_tile(0, t, P if t < NW - 1 else NT_LAST)
            emit_collectives(0)

            nc.gpsimd.load_library(mlp_lib)

            inst_q = [0]

            def emit_group_gathers(layer, wg):
                """Gather + onehot + reduce matmuls for one window group."""
                wsz = min(WG, NW - wg * WG)
                aggws = {}
                psums = {}
                for c in range(N_CHUNKS):
                    g0 = (wg * (N_CHUNKS * WG) + c * WG) * B  # first tile of block group
                    gtiles = WG * B  # tiles incl ghost windows (gathered, unused)
                    rtiles = wsz * B
                    # gather instructions over the real tiles
                    j = 0
                    while j < rtiles:
                        tiles_this = min(8, rtiles - j)
                        ni = tiles_this * P
                        gb = gpool.tile([128, 8, 2 * D], bf16, tag="gb",
                                        name=f"gb{layer}_{wg}_{c}_{j}")
                        off = (g0 + j) * 8
                        nc.gpsimd.dma_gather(
                            gb[:, :tiles_this, :], m_chunks[layer][c][:],
                            gidx_sb[:, off:off + ni // 16],
                            ni, ni, 2 * D, queue_num=inst_q[0] % 4,
                        )
                        inst_q[0] += 1
                        for g in range(0, tiles_this, 4):
                            gsz = min(4, tiles_this - g)
                            oht = opool.tile([128, 4 * P], bf16, tag="oh",
                                             name=f"oh{layer}_{wg}_{c}_{j}_{g}")
                            gt = g0 + j + g
                            nc.vector.tensor_tensor(
                                out=oht[:].rearrange("p (a b) -> p a b", a=4)[:, :gsz, :],
                                in0=dcol_sb[:, gt:gt + gsz].to_broadcast([128, gsz, P]),
                                in1=iota4_sb[:].rearrange("p (a b) -> p a b", a=4)[:, :gsz, :],
                                op=mybir.AluOpType.is_equal,
                            )
                            for u in range(gsz):
                                tt = j + g + u        # tile within this (c, wg) group
                                wi = tt // B          # window within group
                                tib = tt % B
                                w = wg * WG + wi
                                if c == 0 and tib == 0:
                                    psums[wi] = rpool.tile(
                                        [64, P], f32, tag="red",
                                        name=f"psr{layer}_{wg}_{wi}")
                                nc.tensor.matmul(
                                    psums[wi][:], lhsT=gb[:, g + u, 0:D],
                                    rhs=oht[:, u * P:(u + 1) * P],
                                    start=(c == 0 and tib == 0),
                                    stop=(c == N_CHUNKS - 1 and tib == B - 1),
                                )
                                if c == N_CHUNKS - 1 and tib == B - 1:
                                    aggw = apool.tile([64, P], f32, tag="aggw",
                                                      name=f"aggw{layer}_{w}")
                                    nc.scalar.activation(aggw[:], psums[wi][:],
                                                         AF.Copy)
                                    aggws[wi] = aggw
                        j += tiles_this
                return aggws

            def emit_gru_tile(layer, t, aggw):
                sl = slice(t * P, (t + 1) * P)
                psA = upool.tile([128, 3 * D], f32, tag="gru",
                                 name=f"psA{layer}_{t}")
                nc.tensor.matmul(psA[:], lhsT=aggw[:], rhs=wihT_sb[:],
                                 start=True, stop=False)
                nc.tensor.matmul(psA[:, 0:2 * D], lhsT=hT[:, sl],
                                 rhs=whhT_sb[:, 0:2 * D], start=False, stop=False)
                nc.tensor.matmul(psA[:], lhsT=ones1_sb[:], rhs=biasA_sb[:],
                                 start=False, stop=True)
                psB = upool.tile([128, D], f32, tag="gru", name=f"psB{layer}_{t}")
                nc.tensor.matmul(psB[:], lhsT=hT[:, sl],
                                 rhs=whhT_sb[:, 2 * D:3 * D], start=True, stop=False)
                nc.tensor.matmul(psB[:], lhsT=ones1_sb[:], rhs=biasB_sb[:],
                                 start=False, stop=True)
                r = tpool.tile([128, D], f32, tag="r", name=f"r{layer}_{t}")
                z = tpool.tile([128, D], f32, tag="z", name=f"z{layer}_{t}")
                nn = tpool.tile([128, D], f32, tag="nn", name=f"nn{layer}_{t}")
                t1 = tpool.tile([128, D], f32, tag="t1", name=f"t1{layer}_{t}")
                nc.scalar.activation(r[:], psA[:, 0:D], AF.Sigmoid)
                nc.scalar.activation(z[:], psA[:, D:2 * D], AF.Sigmoid)
                nc.vector.tensor_tensor(out=t1[:], in0=r[:], in1=psB[:],
                                        op=mybir.AluOpType.mult)
                nc.vector.tensor_tensor(out=t1[:], in0=t1[:],
                                        in1=psA[:, 2 * D:3 * D],
                                        op=mybir.AluOpType.add)
                nc.scalar.activation(nn[:], t1[:], AF.Tanh)
                nc.vector.tensor_tensor(out=t1[:], in0=h_row[:, t, :], in1=nn[:],
                                        op=mybir.AluOpType.subtract)
                nc.vector.tensor_tensor(out=t1[:], in0=z[:], in1=t1[:],
                                        op=mybir.AluOpType.mult)
                nc.vector.tensor_tensor(out=h_row[:, t, :], in0=nn[:], in1=t1[:],
                                        op=mybir.AluOpType.add)
                if layer < n_layers - 1:
                    pst = upool.tile([64, P], f32, tag="gru",
                                     name=f"pst{layer}_{t}")
                    nc.tensor.transpose(pst[:], h_row[:, t, :], ident_sb[:])
                    nc.scalar.activation(hT[:, sl], pst[:], AF.Copy)
                    emit_m_tile(layer + 1, t, P if t < NW - 1 else NT_LAST)

            for layer in range(n_layers):
                for wg in range(NWG):
                    aggws = emit_group_gathers(layer, wg)
                    wsz = min(WG, NW - wg * WG)
                    for wi in range(wsz):
                        emit_gru_tile(layer, wg * WG + wi, aggws[wi])
                if layer < n_layers - 1:
                    emit_collectives(layer + 1)

            # ---- pooling ----
            psP0 = upool.tile([128, D], f32, tag="gru", name="psP0")
            psP1 = upool.tile([128, D], f32, tag="gru", name="psP1")
            for t in range(NW):
                oh0 = opool.tile([128, 4 * P], f32, tag="oh", name=f"ohp{t}")
                nc.vector.tensor_tensor(
                    out=oh0[:, 0:P],
                    in0=bc0_sb[:, t:t + 1].to_broadcast([128, P]),
                    in1=iotaf_sb[:], op=mybir.AluOpType.is_equal)
                nc.vector.tensor_tensor(
                    out=oh0[:, P:2 * P],
                    in0=bc1_sb[:, t:t + 1].to_broadcast([128, P]),
                    in1=iotaf_sb[:], op=mybir.AluOpType.is_equal)
                nc.tensor.matmul(psP0[:], lhsT=oh0[:, 0:P], rhs=h_row[:, t, :],
                                 start=(t == 0), stop=(t == NW - 1))
                nc.tensor.matmul(psP1[:], lhsT=oh0[:, P:2 * P], rhs=h_row[:, t, :],
                                 start=(t == 0), stop=(t == NW - 1))
            po = tpool.tile([128, D], f32, tag="po", name="po")
            nc.scalar.activation(po[:], psP0[:], AF.Copy)
            nc.sync.dma_start(pooled[0:128, :], po[:])
            po2 = tpool.tile([128, D], f32, tag="po", name="po2")
            nc.scalar.activation(po2[:], psP1[:], AF.Copy)
            nc.sync.dma_start(pooled[128:256, :], po2[:])

    nc.compile()
    return nc


def kernel(x, edge_index, batch, weight, W_ih, W_hh, b_ih, b_hh,
           _trace=False):
    from concourse.bass_utils import run_bass_kernel_spmd

    x = np.asarray(x, np.float32)
    weight = np.asarray(weight, np.float32)
    W_ih = np.asarray(W_ih, np.float32)
    W_hh = np.asarray(W_hh, np.float32)
    b_ih = np.asarray(b_ih, np.float32)
    b_hh = np.asarray(b_hh, np.float32)
    N, D = x.shape
    n_layers = weight.shape[0]
    NPC = N // N_CORES

    meta = _host_prep(x, edge_index, batch)
    key = (N, D, n_layers, meta["B"])
    if key not in _cache:
        _cache[key] = _build_program(meta, n_layers)
    nc = _cache[key]

    iota_b = np.tile(np.arange(P, dtype=np.float32),
                     (128, 4)).astype(ml_dtypes.bfloat16)
    iota_f = np.tile(np.arange(P, dtype=np.float32), (128, 1))
    ident_np = np.eye(P, dtype=np.float32)
    ones1_np = np.ones((1, P), np.float32)
    wmat_np = np.concatenate([weight[i] for i in range(n_layers)], axis=1)
    wihT_np = np.ascontiguousarray(W_ih.T)
    whhT_np = np.ascontiguousarray(W_hh.T)
    biasA_np = np.concatenate([
        b_ih[0:D] + b_hh[0:D], b_ih[D:2 * D] + b_hh[D:2 * D],
        b_ih[2 * D:3 * D]]).reshape(1, 3 * D).astype(np.float32)
    biasB_np = b_hh[2 * D:3 * D].reshape(1, D).astype(np.float32)

    in_maps = []
    for k in range(N_CORES):
        ck = meta["cores"][k]
        in_maps.append(dict(
            xs=np.ascontiguousarray(x[k * NPC:(k + 1) * NPC]),
            gidx=ck["gidx"], dcol=ck["dcol"], bc0=ck["bc0"], bc1=ck["bc1"],
            iota4=iota_b, iotaf=iota_f, ident=ident_np, ones1=ones1_np,
            wmat=wmat_np, wihT=wihT_np, whhT=whhT_np, biasA=biasA_np,
            biasB=biasB_np,
        ))

    res = run_bass_kernel_spmd(nc, in_maps, core_ids=list(range(N_CORES)),
                               trace=_trace)
    out = np.zeros((256, D), np.float32)
    for k in range(N_CORES):
        out += res.results[k]["pooled"]
    kernel._last_exec_time_ns = res.exec_time_ns
    kernel._last_result = res
    return out



# revision 20
# speedup vs baseline: 1.3027x; 1.3027x over previous
"""GNN message-passing encoder (GatedGraphConv-style) on 8 Trainium2 NeuronCores.

Strategy (dst-sharded, gather-only, scatter-free), v2:
  - Nodes are partitioned across 8 cores (12500 rows each). Within a core,
    nodes are host-permuted into 98 windows of 128 slots, balancing total
    in-degree per window. The global node order is quarter-major over
    (quarter, core, position) so each of 4 AllGathers delivers one contiguous
    25000-row slice of a single [100000, 128] bf16 m-table per layer.
  - Per layer every core computes m = h @ W for its slice (duplicated to
    256 B rows), bounces it to DRAM, and 4 per-quarter AllGathers assemble the
    full table. Collectives fire as soon as their quarter's m rows exist, so
    they overlap the same layer's remaining compute.
  - Edges are grouped per (chunk, window) where a "chunk" is one of 4
    overlapping 32768-row windows into the table (bases 0/25000/47232/67232),
    sized so gpsimd.dma_gather's int16 indices reach every source. Each
    window's edges are water-filled into the 4 chunks under fixed tile caps
    (5,4,4,4); host-side node rebalancing makes this feasible, cutting pad
    slots vs a uniform per-block cap.
  - The per-core edge stream is window-group-major: groups of WG=4 dst
    windows, within a group the 4 chunks back to back, block (c,w) padded to
    exactly CAPS[c] tiles of 128 slots, so the SPMD program is identical on
    all cores.
  - The scatter-add (segment-sum over dst) is TensorE matmuls with data-built
    one-hot selection matrices (is_equal of a per-slot dst-column stream
    against an iota row). All tiles of one window accumulate into one PSUM
    bank; ScalarE copies the finished window into SBUF. Pad slots carry a
    sentinel column so they match nothing.
  - The GRU cell (PE matmuls + ACT sigmoid/tanh + DVE elementwise) for a
    window group runs while the next group is being gathered; the next
    layer's m tile is produced right after each GRU tile. Bias matmuls are
    skipped when the GRU biases are all-zero. Graph pooling uses the same
    onehot-matmul trick over the batch vector; each core emits a [256, 64]
    partial pooled sum and the host adds the 8 partials.
"""

import sys

for _p in ("/opt/trn_rl_repo", "/root/.axon_site/_ro/trn_rl_repo"):
    if _p not in sys.path:
        sys.path.insert(0, _p)

import numpy as np
import ml_dtypes

P = 128
N_CORES = 8
N_CHUNKS = 4
WG = 4                 # windows per group
PAD_SENTINEL = 999.0
SPAN = 4096            # rows each core contributes to one chunk table
# (spans, caps): chunk c covers core-local positions [spans[c], spans[c]+SPAN);
# block (c, w) gets caps[c] tiles of 128 slots in the edge stream.
CONFIGS = (
    ((0, 2800, 6500, 8404), (5, 4, 4, 4)),
    ((0, 2800, 5600, 8404), (5, 4, 4, 5)),
    ((0, 2800, 5600, 8404), (5, 5, 5, 5)),
)

_cache = {}


def _wrap16(idx, channels=128):
    n = len(idx)
    a = np.asarray(idx, np.int16).reshape(n // 16, 16).T
    return np.ascontiguousarray(np.tile(a, (channels // 16, 1)))


def _balance_windows(nd, nwin):
    """Assign len(nd) nodes to nwin windows of <=128 slots, balancing sum."""
    npc = len(nd)
    cap = np.full(nwin, P)
    cap[-1] = npc - (nwin - 1) * P
    order = np.argsort(-nd, kind="stable")
    loads = np.zeros(nwin)
    cnts = np.zeros(nwin, dtype=np.int64)
    assign = np.zeros(npc, dtype=np.int64)
    for i in order:
        elig = np.where(cnts < cap)[0]
        b = elig[np.argmin(loads[elig])]
        assign[i] = b
        loads[b] += nd[i]
        cnts[b] += 1
    return assign


def _water_fill(sv, spans, caps, scale):
    """Split sorted composite src keys (pos*scale + core) into 4 chunks.

    Chunk c accepts positions in [spans[c], spans[c]+SPAN). Must-take edges
    (not coverable by a later chunk) first, then smallest flexible positions.
    Returns None if infeasible.
    """
    out = []
    rem = np.sort(sv)
    for c in range(N_CHUNKS):
        lo, hi = spans[c] * scale, (spans[c] + SPAN) * scale
        nxt = spans[c + 1] * scale if c < N_CHUNKS - 1 else 10 ** 12
        must = rem[(rem >= lo) & (rem < nxt)]
        flex = rem[(rem >= nxt) & (rem < hi)]
        if len(must) > caps[c] * P:
            return None
        tk = min(caps[c] * P - len(must), len(flex))
        out.append(np.concatenate([must, flex[:tk]]))
        rem = np.concatenate([flex[tk:], rem[rem >= hi]])
    if len(rem):
        return None
    return out


def _host_prep(x, edge_index, batch):
    N, D = x.shape
    NPC = N // N_CORES
    NW = (NPC + P - 1) // P
    NWG = (NW + WG - 1) // WG

    src = np.asarray(edge_index[0], np.int64)
    dst = np.asarray(edge_index[1], np.int64)
    batch = np.asarray(batch, np.int64)
    deg = np.bincount(dst, minlength=N)

    # per-core permutation: node -> (window, slot), degree-balanced
    node_at_pos = np.zeros((N_CORES, NPC), dtype=np.int64)   # local node at pos
    localpos = np.zeros(N, dtype=np.int64)
    for k in range(N_CORES):
        nd = deg[k * NPC:(k + 1) * NPC]
        assign = _balance_windows(nd, NW)
        order = np.argsort(assign, kind="stable")
        node_at_pos[k] = order
        localpos[k * NPC + order] = np.arange(NPC)

    src_core = src // NPC
    src_j = localpos[src]                 # core-local position of the source
    dst_core = dst // NPC

    per_core_edges = []
    for k in range(N_CORES):
        sel = dst_core == k
        # composite key preserves position ordering, keeps src core recoverable
        sv = src_j[sel] * N_CORES + src_core[sel]
        per_core_edges.append((sv, localpos[dst[sel]]))

    # choose (spans, caps): first config feasible for every (core, window)
    spans = caps = None
    plans = None
    for cand_spans, cand_caps in CONFIGS:
        plans = []
        ok = True
        for k in range(N_CORES):
            sv, pd = per_core_edges[k]
            w = pd // P
            plan_k = []
            for wwin in range(NW):
                split = _water_fill(sv[w == wwin], cand_spans, cand_caps,
                                    N_CORES)
                if split is None:
                    ok = False
                    break
                plan_k.append(split)
            if not ok:
                break
            plans.append(plan_k)
        if ok:
            spans, caps = cand_spans, cand_caps
            break
    assert caps is not None, "edge->chunk water-filling infeasible"

    # stream layout: for wg: for c: for w in group: caps[c] tiles of 128
    tiles_per_group = []
    block_start = {}
    tb = 0
    for wg in range(NWG):
        wsz = min(WG, NW - wg * WG)
        for c in range(N_CHUNKS):
            block_start[(c, wg)] = tb
            tb += wsz * caps[c]
    S_tiles = tb
    S = S_tiles * P

    cores = []
    for k in range(N_CORES):
        sv_all, pd_all = per_core_edges[k]
        w_all = pd_all // P
        slot_all = pd_all % P
        gidx = np.zeros(S, np.int16)
        dcol = np.full(S, PAD_SENTINEL, np.float32)
        for wwin in range(NW):
            m = w_all == wwin
            sv = sv_all[m]
            slots = slot_all[m]
            split = plans[k][wwin]
            # assign each edge to its chunk: reproduce water-fill membership
            used = np.zeros(len(sv), dtype=bool)
            order = np.argsort(sv, kind="stable")
            sv_sorted = sv[order]
            slot_sorted = slots[order]
            wg = wwin // WG
            wi = wwin % WG
            for c in range(N_CHUNKS):
                vals = split[c]          # sorted composite keys for this chunk
                idxs = np.searchsorted(sv_sorted, vals)
                base = block_start[(c, wg)] * P + wi * caps[c] * P
                r = 0
                for v, i0 in zip(vals, idxs):
                    i = i0
                    while used[i] or sv_sorted[i] != v:
                        i += 1
                    used[i] = True
                    pos = base + r
                    # table row: src_core*SPAN + (src_pos - spans[c])
                    gidx[pos] = (v % N_CORES) * SPAN + (v // N_CORES - spans[c])
                    dcol[pos] = slot_sorted[i]
                    r += 1
            assert used.all()
        bpos = batch[k * NPC + node_at_pos[k]]
        bc0 = np.full(NW * P, PAD_SENTINEL, np.float32)
        bc1 = np.full(NW * P, PAD_SENTINEL, np.float32)
        bc0[:NPC] = np.where(bpos < 128, bpos, PAD_SENTINEL)
        bc1[:NPC] = np.where(bpos >= 128, bpos - 128, PAD_SENTINEL)
        # x in (slot, window) grid layout: one contiguous DMA into h_row/hT
        xk = x[k * NPC:(k + 1) * NPC][node_at_pos[k]].astype(np.float32)
        xg = np.zeros((NW * P, D), np.float32)
        xg[:NPC] = xk
        xs_grid = np.ascontiguousarray(
            xg.reshape(NW, P, D).transpose(1, 0, 2).reshape(P, NW * D))
        xsT_grid = np.ascontiguousarray(xg.T)          # [D, NW*P]
        cores.append(dict(
            gidx=_wrap16(gidx),
            dcol=np.ascontiguousarray(
                dcol.reshape(S // P, P).T.astype(ml_dtypes.bfloat16)),
            bc0=np.ascontiguousarray(bc0.reshape(NW, P).T),
            bc1=np.ascontiguousarray(bc1.reshape(NW, P).T),
            perm=node_at_pos[k], xs=xs_grid, xsT=xsT_grid,
        ))
    return dict(N=N, D=D, NPC=NPC, NW=NW, NWG=NWG, spans=spans, caps=caps,
                S=S, S_tiles=S_tiles, block_start=block_start, cores=cores)


def _build_program(meta, n_layers, has_bias):
    import concourse.bacc as bacc
    import concourse.mybir as mybir
    import concourse.tile as tile
    from concourse.library_config import mlp as mlp_lib

    N, D, NPC, NW, NWG, S = (meta[z] for z in
                             ("N", "D", "NPC", "NW", "NWG", "S"))
    caps = meta["caps"]
    spans = meta["spans"]
    block_start = meta["block_start"]
    NT_LAST = NPC - (NW - 1) * P
    f32 = mybir.dt.float32
    bf16 = mybir.dt.bfloat16
    AF = mybir.ActivationFunctionType

    nc = bacc.Bacc("TRN2", target_bir_lowering=False, debug=False,
                   num_swdge_queues=4)

    xs = nc.dram_tensor("xs", [128, NW * D], f32, kind="ExternalInput")
    xsT = nc.dram_tensor("xsT", [D, NW * P], f32, kind="ExternalInput")
    gidx = nc.dram_tensor("gidx", [128, S // 16], mybir.dt.int16, kind="ExternalInput")
    dcol = nc.dram_tensor("dcol", [128, S // P], bf16, kind="ExternalInput")
    bc0 = nc.dram_tensor("bc0", [128, NW], f32, kind="ExternalInput")
    bc1 = nc.dram_tensor("bc1", [128, NW], f32, kind="ExternalInput")
    iota8 = nc.dram_tensor("iota8", [128, 8 * P], bf16, kind="ExternalInput")
    iotaf = nc.dram_tensor("iotaf", [128, P], f32, kind="ExternalInput")
    ident = nc.dram_tensor("ident", [128, P], f32, kind="ExternalInput")
    wmat = nc.dram_tensor("wmat", [D, n_layers * D], f32, kind="ExternalInput")
    wihT = nc.dram_tensor("wihT", [D, 3 * D], f32, kind="ExternalInput")
    whhT = nc.dram_tensor("whhT", [D, 3 * D], f32, kind="ExternalInput")
    if has_bias:
        ones1 = nc.dram_tensor("ones1", [1, P], f32, kind="ExternalInput")
        biasA = nc.dram_tensor("biasA", [1, 3 * D], f32, kind="ExternalInput")
        biasB = nc.dram_tensor("biasB", [1, D], f32, kind="ExternalInput")
    pooled = nc.dram_tensor("pooled", [256, D], f32, kind="ExternalOutput")

    # fire collective c for a layer once window t covers rows < spans[c]+SPAN
    qfire = {}
    for c in range(N_CHUNKS):
        qfire[min(-(-(spans[c] + SPAN) // P), NW) - 1] = c

    with tile.TileContext(nc) as tc:
        with (
            tc.tile_pool(name="const", bufs=1) as cpool,
            tc.tile_pool(name="state", bufs=1) as spool,
            tc.tile_pool(name="gbuf", bufs=14) as gpool,
            tc.tile_pool(name="oh", bufs=8) as opool,
            tc.tile_pool(name="aggw", bufs=8) as apool,
            tc.tile_pool(name="tmp", bufs=8) as tpool,
            tc.tile_pool(name="red", bufs=4, space="PSUM") as rpool,
            tc.tile_pool(name="gru", bufs=4, space="PSUM") as upool,
            tc.tile_pool(name="dram", bufs=1, space="DRAM") as dpool,
        ):
            gidx_sb = spool.tile([128, S // 16], mybir.dt.int16, tag="gidx")
            dcol_sb = spool.tile([128, S // P], bf16, tag="dcol")
            h_row = spool.tile([128, NW, D], f32, tag="hrow")
            hT = spool.tile([64, NW * P], f32, tag="hT")
            iota8_sb = cpool.tile([128, 8 * P], bf16, tag="iota8")
            iotaf_sb = cpool.tile([128, P], f32, tag="iotaf")
            ident_sb = cpool.tile([128, P], f32, tag="ident")
            wmat_sb = cpool.tile([D, n_layers * D], f32, tag="wmat")
            wihT_sb = cpool.tile([D, 3 * D], f32, tag="wihT")
            whhT_sb = cpool.tile([D, 3 * D], f32, tag="whhT")
            bc0_sb = cpool.tile([128, NW], f32, tag="bc0")
            bc1_sb = cpool.tile([128, NW], f32, tag="bc1")

            nc.sync.dma_start(gidx_sb[:], gidx[:])
            nc.sync.dma_start(dcol_sb[:], dcol[:])
            nc.sync.dma_start(iota8_sb[:], iota8[:])
            nc.sync.dma_start(iotaf_sb[:], iotaf[:])
            nc.sync.dma_start(ident_sb[:], ident[:])
            nc.sync.dma_start(wmat_sb[:], wmat[:])
            nc.sync.dma_start(wihT_sb[:], wihT[:])
            nc.sync.dma_start(whhT_sb[:], whhT[:])
            nc.sync.dma_start(bc0_sb[:], bc0[:])
            nc.sync.dma_start(bc1_sb[:], bc1[:])
            if has_bias:
                ones1_sb = cpool.tile([1, P], f32, tag="ones1")
                biasA_sb = cpool.tile([1, 3 * D], f32, tag="biasA")
                biasB_sb = cpool.tile([1, D], f32, tag="biasB")
                nc.sync.dma_start(ones1_sb[:], ones1[:])
                nc.sync.dma_start(biasA_sb[:], biasA[:])
                nc.sync.dma_start(biasB_sb[:], biasB[:])

            m_bounces = []
            m_tables = []
            for layer in range(n_layers):
                mb = dpool.tile([NPC, 2 * D], bf16, tag=f"mb{layer}", name=f"mb{layer}")
                tabs = [dpool.tile([N_CORES * SPAN, 2 * D], bf16,
                                   addr_space="Shared", tag=f"tab{layer}_{c}",
                                   name=f"tab{layer}_{c}")
                        for c in range(N_CHUNKS)]
                m_bounces.append(mb)
                m_tables.append(tabs)

            def emit_m_tile(layer, t, rows):
                psm = upool.tile([128, D], f32, tag="gru", name=f"psm{layer}_{t}")
                nc.tensor.matmul(psm[:], lhsT=hT[:, t * P:(t + 1) * P],
                                 rhs=wmat_sb[:, layer * D:(layer + 1) * D],
                                 start=True, stop=True)
                mt = tpool.tile([128, 2 * D], bf16, tag="mt", name=f"mt{layer}_{t}")
                nc.scalar.activation(mt[:, 0:D], psm[:], AF.Copy)
                nc.scalar.activation(mt[:, D:2 * D], psm[:], AF.Copy)
                nc.sync.dma_start(
                    m_bounces[layer][t * P:t * P + rows, :], mt[:rows, :])

            def emit_collective(layer, c):
                nc.gpsimd.collective_compute(
                    "AllGather", mybir.AluOpType.bypass,
                    ins=[m_bounces[layer][spans[c]:spans[c] + SPAN, :]],
                    outs=[m_tables[layer][c][:]],
                    replica_groups=[list(range(N_CORES))],
                )

            # ---- init: load x -> h_row/hT (pre-transposed), layer-0 m ----
            nc.sync.dma_start(
                h_row[:], xs[:].rearrange("p (t d) -> p t d", d=D))
            nc.sync.dma_start(hT[:], xsT[:])
            for t in range(NW):
                emit_m_tile(0, t, P if t < NW - 1 else NT_LAST)
                if t in qfire:
                    emit_collective(0, qfire[t])

            nc.gpsimd.load_library(mlp_lib)

            inst_q = [0]

            def emit_group_gathers(layer, wg):
                """Gather + onehot + reduce matmuls for one window group."""
                wsz = min(WG, NW - wg * WG)
                aggws = {}
                psums = {}
                for c in range(N_CHUNKS):
                    g0 = block_start[(c, wg)]
                    rtiles = wsz * caps[c]
                    tab_win = m_tables[layer][c][:]
                    j = 0
                    while j < rtiles:
                        tiles_this = min(8, rtiles - j)
                        ni = tiles_this * P
                        gb = gpool.tile([128, 8, 2 * D], bf16, tag="gb",
                                        name=f"gb{layer}_{wg}_{c}_{j}")
                        off = (g0 + j) * 8
                        nc.gpsimd.dma_gather(
                            gb[:, :tiles_this, :], tab_win,
                            gidx_sb[:, off:off + ni // 16],
                            ni, ni, 2 * D, queue_num=inst_q[0] % 4,
                        )
                        inst_q[0] += 1
                        gt = g0 + j
                        oht = opool.tile([128, 8, P], bf16, tag="oh",
                                         name=f"oh{layer}_{wg}_{c}_{j}")
                        nc.vector.tensor_tensor(
                            out=oht[:, :tiles_this, :],
                            in0=dcol_sb[:, gt:gt + tiles_this].to_broadcast(
                                [128, tiles_this, P]),
                            in1=iota8_sb[:].rearrange(
                                "p (a b) -> p a b", a=8)[:, :tiles_this, :],
                            op=mybir.AluOpType.is_equal,
                        )
                        for u in range(tiles_this):
                            tt = j + u
                            wi = tt // caps[c]
                            tib = tt % caps[c]
                            w = wg * WG + wi
                            if c == 0 and tib == 0:
                                psums[wi] = rpool.tile(
                                    [64, P], f32, tag="red",
                                    name=f"psr{layer}_{wg}_{wi}")
                            nc.tensor.matmul(
                                psums[wi][:], lhsT=gb[:, u, 0:D],
                                rhs=oht[:, u, :],
                                start=(c == 0 and tib == 0),
                                stop=(c == N_CHUNKS - 1 and tib == caps[c] - 1),
                            )
                            if c == N_CHUNKS - 1 and tib == caps[c] - 1:
                                aggw = apool.tile([64, P], f32, tag="aggw",
                                                  name=f"aggw{layer}_{w}")
                                nc.scalar.activation(aggw[:], psums[wi][:],
                                                     AF.Copy)
                                aggws[wi] = aggw
                        j += tiles_this
                return aggws

            def emit_gru_tile(layer, t, aggw):
                sl = slice(t * P, (t + 1) * P)
                psA = upool.tile([128, 3 * D], f32, tag="gru",
                                 name=f"psA{layer}_{t}")
                nc.tensor.matmul(psA[:, 0:2 * D], lhsT=hT[:, sl],
                                 rhs=whhT_sb[:, 0:2 * D], start=True, stop=False)
                nc.tensor.matmul(psA[:], lhsT=aggw[:], rhs=wihT_sb[:],
                                 start=False, stop=not has_bias)
                if has_bias:
                    nc.tensor.matmul(psA[:], lhsT=ones1_sb[:], rhs=biasA_sb[:],
                                     start=False, stop=True)
                psB = upool.tile([128, D], f32, tag="gru", name=f"psB{layer}_{t}")
                nc.tensor.matmul(psB[:], lhsT=hT[:, sl],
                                 rhs=whhT_sb[:, 2 * D:3 * D], start=True,
                                 stop=not has_bias)
                if has_bias:
                    nc.tensor.matmul(psB[:], lhsT=ones1_sb[:], rhs=biasB_sb[:],
                                     start=False, stop=True)
                rz = tpool.tile([128, 2 * D], f32, tag="rz", name=f"rz{layer}_{t}")
                nn = tpool.tile([128, D], f32, tag="nn", name=f"nn{layer}_{t}")
                t1 = tpool.tile([128, D], f32, tag="t1", name=f"t1{layer}_{t}")
                nc.scalar.activation(rz[:], psA[:, 0:2 * D], AF.Sigmoid)
                nc.vector.tensor_tensor(out=t1[:], in0=rz[:, 0:D], in1=psB[:],
                                        op=mybir.AluOpType.mult)
                nc.vector.tensor_tensor(out=t1[:], in0=t1[:],
                                        in1=psA[:, 2 * D:3 * D],
                                        op=mybir.AluOpType.add)
                nc.scalar.activation(nn[:], t1[:], AF.Tanh)
                nc.vector.tensor_tensor(out=t1[:], in0=h_row[:, t, :], in1=nn[:],
                                        op=mybir.AluOpType.subtract)
                nc.vector.tensor_tensor(out=t1[:], in0=rz[:, D:2 * D], in1=t1[:],
                                        op=mybir.AluOpType.mult)
                nc.vector.tensor_tensor(out=h_row[:, t, :], in0=nn[:], in1=t1[:],
                                        op=mybir.AluOpType.add)
                if layer < n_layers - 1:
                    pst = upool.tile([64, P], f32, tag="gru",
                                     name=f"pst{layer}_{t}")
                    nc.tensor.transpose(pst[:], h_row[:, t, :], ident_sb[:])
                    nc.scalar.activation(hT[:, sl], pst[:], AF.Copy)
                    emit_m_tile(layer + 1, t, P if t < NW - 1 else NT_LAST)

            for layer in range(n_layers):
                for wg in range(NWG):
                    aggws = emit_group_gathers(layer, wg)
                    wsz = min(WG, NW - wg * WG)
                    for wi in range(wsz):
                        t = wg * WG + wi
                        emit_gru_tile(layer, t, aggws[wi])
                        if layer < n_layers - 1 and t in qfire:
                            emit_collective(layer + 1, qfire[t])

            # ---- pooling ----
            psP0 = upool.tile([128, D], f32, tag="gru", name="psP0")
            psP1 = upool.tile([128, D], f32, tag="gru", name="psP1")
            for t in range(NW):
                oh0 = opool.tile([128, 8, P], f32, tag="oh", name=f"ohp{t}")
                nc.vector.tensor_tensor(
                    out=oh0[:, 0, :],
                    in0=bc0_sb[:, t:t + 1].to_broadcast([128, P]),
                    in1=iotaf_sb[:], op=mybir.AluOpType.is_equal)
                nc.vector.tensor_tensor(
                    out=oh0[:, 1, :],
                    in0=bc1_sb[:, t:t + 1].to_broadcast([128, P]),
                    in1=iotaf_sb[:], op=mybir.AluOpType.is_equal)
                nc.tensor.matmul(psP0[:], lhsT=oh0[:, 0, :], rhs=h_row[:, t, :],
                                 start=(t == 0), stop=(t == NW - 1))
                nc.tensor.matmul(psP1[:], lhsT=oh0[:, 1, :], rhs=h_row[:, t, :],
                                 start=(t == 0), stop=(t == NW - 1))
            po = tpool.tile([128, D], f32, tag="po", name="po")
            nc.scalar.activation(po[:], psP0[:], AF.Copy)
            nc.sync.dma_start(pooled[0:128, :], po[:])
            po2 = tpool.tile([128, D], f32, tag="po", name="po2")
            nc.scalar.activation(po2[:], psP1[:], AF.Copy)
            nc.sync.dma_start(pooled[128:256, :], po2[:])

    nc.compile()
    return nc


def kernel(x, edge_index, batch, weight, W_ih, W_hh, b_ih, b_hh,
           _trace=False):
    from concourse.bass_utils import run_bass_kernel_spmd

    x = np.asarray(x, np.float32)
    weight = np.asarray(weight, np.float32)
    W_ih = np.asarray(W_ih, np.float32)
    W_hh = np.asarray(W_hh, np.float32)
    b_ih = np.asarray(b_ih, np.float32)
    b_hh = np.asarray(b_hh, np.float32)
    N, D = x.shape
    n_layers = weight.shape[0]
    NPC = N // N_CORES

    meta = _host_prep(x, edge_index, batch)
    has_bias = bool(np.any(b_ih) or np.any(b_hh))
    key = (N, D, n_layers, meta["spans"], meta["caps"], has_bias)
    if key not in _cache:
        _cache[key] = _build_program(meta, n_layers, has_bias)
    nc = _cache[key]

    iota8_np = np.tile(np.arange(P, dtype=np.float32),
                       (128, 8)).astype(ml_dtypes.bfloat16)
    iota_f = np.tile(np.arange(P, dtype=np.float32), (128, 1))
    ident_np = np.eye(P, dtype=np.float32)
    wmat_np = np.concatenate([weight[i] for i in range(n_layers)], axis=1)
    wihT_np = np.ascontiguousarray(W_ih.T)
    whhT_np = np.ascontiguousarray(W_hh.T)

    in_maps = []
    for k in range(N_CORES):
        ck = meta["cores"][k]
        im = dict(
            xs=ck["xs"], xsT=ck["xsT"],
            gidx=ck["gidx"], dcol=ck["dcol"], bc0=ck["bc0"], bc1=ck["bc1"],
            iota8=iota8_np, iotaf=iota_f, ident=ident_np,
            wmat=wmat_np, wihT=wihT_np, whhT=whhT_np,
        )
        if has_bias:
            im["ones1"] = np.ones((1, P), np.float32)
            im["biasA"] = np.concatenate([
                b_ih[0:D] + b_hh[0:D], b_ih[D:2 * D] + b_hh[D:2 * D],
                b_ih[2 * D:3 * D]]).reshape(1, 3 * D).astype(np.float32)
            im["biasB"] = b_hh[2 * D:3 * D].reshape(1, D).astype(np.float32)
        in_maps.append(im)

    res = run_bass_kernel_spmd(nc, in_maps, core_ids=list(range(N_CORES)),
                               trace=_trace)
    out = np.zeros((256, D), np.float32)
    for k in range(N_CORES):
        out += res.results[k]["pooled"]
    kernel._last_exec_time_ns = res.exec_time_ns
    kernel._last_result = res
    return out


# revision 45
# speedup vs baseline: 1.4412x; 1.1064x over previous
"""GNN message-passing encoder (GatedGraphConv-style) on 8 Trainium2 NeuronCores.

Strategy (dst-sharded, gather-only, scatter-free), v2:
  - Nodes are partitioned across 8 cores (12500 rows each). Within a core,
    nodes are host-permuted into 98 windows of 128 slots, balancing total
    in-degree per window. The global node order is quarter-major over
    (quarter, core, position) so each of 4 AllGathers delivers one contiguous
    25000-row slice of a single [100000, 128] bf16 m-table per layer.
  - Per layer every core computes m = h @ W for its slice (duplicated to
    256 B rows), bounces it to DRAM, and 4 per-quarter AllGathers assemble the
    full table. Collectives fire as soon as their quarter's m rows exist, so
    they overlap the same layer's remaining compute.
  - Edges are grouped per (chunk, window) where a "chunk" is one of 4
    overlapping 32768-row windows into the table (bases 0/25000/47232/67232),
    sized so gpsimd.dma_gather's int16 indices reach every source. Each
    window's edges are water-filled into the 4 chunks under fixed tile caps
    (5,4,4,4); host-side node rebalancing makes this feasible, cutting pad
    slots vs a uniform per-block cap.
  - The per-core edge stream is window-group-major: groups of WG=4 dst
    windows, within a group the 4 chunks back to back, block (c,w) padded to
    exactly CAPS[c] tiles of 128 slots, so the SPMD program is identical on
    all cores.
  - The scatter-add (segment-sum over dst) is TensorE matmuls with data-built
    one-hot selection matrices (is_equal of a per-slot dst-column stream
    against an iota row). All tiles of one window accumulate into one PSUM
    bank; ScalarE copies the finished window into SBUF. Pad slots carry a
    sentinel column so they match nothing.
  - The GRU cell (PE matmuls + ACT sigmoid/tanh + DVE elementwise) for a
    window group runs while the next group is being gathered; the next
    layer's m tile is produced right after each GRU tile. Bias matmuls are
    skipped when the GRU biases are all-zero. Graph pooling uses the same
    onehot-matmul trick over the batch vector; each core emits a [256, 64]
    partial pooled sum and the host adds the 8 partials.
"""

import sys

for _p in ("/opt/trn_rl_repo", "/root/.axon_site/_ro/trn_rl_repo"):
    if _p not in sys.path:
        sys.path.insert(0, _p)

import numpy as np
import ml_dtypes

P = 128
N_CORES = 8
N_CHUNKS = 4
WG = 4                 # windows per group
PAD_SENTINEL = 999.0
SPAN = 4096            # rows each core contributes to one chunk table
# (spans, caps): chunk c covers core-local positions [spans[c], spans[c]+SPAN);
# block (c, w) gets caps[c] tiles of 128 slots in the edge stream.
CONFIGS = (
    ((0, 2800, 6500, 8404), (5, 4, 4, 4)),
    ((0, 2800, 5600, 8404), (5, 4, 4, 5)),
    ((0, 2800, 5600, 8404), (5, 5, 5, 5)),
)

_cache = {}


def _wrap16(idx, channels=128):
    n = len(idx)
    a = np.asarray(idx, np.int16).reshape(n // 16, 16).T
    return np.ascontiguousarray(np.tile(a, (channels // 16, 1)))


def _balance_windows(nd, nwin):
    """Assign len(nd) nodes to nwin windows of <=128 slots, balancing sum."""
    npc = len(nd)
    cap = np.full(nwin, P)
    cap[-1] = npc - (nwin - 1) * P
    order = np.argsort(-nd, kind="stable")
    loads = np.zeros(nwin)
    cnts = np.zeros(nwin, dtype=np.int64)
    assign = np.zeros(npc, dtype=np.int64)
    for i in order:
        elig = np.where(cnts < cap)[0]
        b = elig[np.argmin(loads[elig])]
        assign[i] = b
        loads[b] += nd[i]
        cnts[b] += 1
    return assign


def _water_fill(sv, spans, caps, scale):
    """Split sorted composite src keys (pos*scale + core) into 4 chunks.

    Chunk c accepts positions in [spans[c], spans[c]+SPAN). Must-take edges
    (not coverable by a later chunk) first, then smallest flexible positions.
    Returns None if infeasible.
    """
    out = []
    rem = np.sort(sv)
    for c in range(N_CHUNKS):
        lo, hi = spans[c] * scale, (spans[c] + SPAN) * scale
        nxt = spans[c + 1] * scale if c < N_CHUNKS - 1 else 10 ** 12
        must = rem[(rem >= lo) & (rem < nxt)]
        flex = rem[(rem >= nxt) & (rem < hi)]
        if len(must) > caps[c] * P:
            return None
        tk = min(caps[c] * P - len(must), len(flex))
        out.append(np.concatenate([must, flex[:tk]]))
        rem = np.concatenate([flex[tk:], rem[rem >= hi]])
    if len(rem):
        return None
    return out


def _host_prep(x, edge_index, batch):
    N, D = x.shape
    NPC = N // N_CORES
    NW = (NPC + P - 1) // P
    NWG = (NW + WG - 1) // WG

    src = np.asarray(edge_index[0], np.int64)
    dst = np.asarray(edge_index[1], np.int64)
    batch = np.asarray(batch, np.int64)
    deg = np.bincount(dst, minlength=N)

    # per-core permutation: node -> (window, slot), degree-balanced
    node_at_pos = np.zeros((N_CORES, NPC), dtype=np.int64)   # local node at pos
    localpos = np.zeros(N, dtype=np.int64)
    for k in range(N_CORES):
        nd = deg[k * NPC:(k + 1) * NPC]
        assign = _balance_windows(nd, NW)
        order = np.argsort(assign, kind="stable")
        node_at_pos[k] = order
        localpos[k * NPC + order] = np.arange(NPC)

    src_core = src // NPC
    src_j = localpos[src]                 # core-local position of the source
    dst_core = dst // NPC

    per_core_edges = []
    for k in range(N_CORES):
        sel = dst_core == k
        # composite key preserves position ordering, keeps src core recoverable
        sv = src_j[sel] * N_CORES + src_core[sel]
        per_core_edges.append((sv, localpos[dst[sel]]))

    # choose (spans, caps): first config feasible for every (core, window)
    spans = caps = None
    plans = None
    for cand_spans, cand_caps in CONFIGS:
        plans = []
        ok = True
        for k in range(N_CORES):
            sv, pd = per_core_edges[k]
            w = pd // P
            plan_k = []
            for wwin in range(NW):
                split = _water_fill(sv[w == wwin], cand_spans, cand_caps,
                                    N_CORES)
                if split is None:
                    ok = False
                    break
                plan_k.append(split)
            if not ok:
                break
            plans.append(plan_k)
        if ok:
            spans, caps = cand_spans, cand_caps
            break
    assert caps is not None, "edge->chunk water-filling infeasible"

    # stream layout: for wg: for c: for w in group: caps[c] tiles of 128
    tiles_per_group = []
    block_start = {}
    tb = 0
    for wg in range(NWG):
        wsz = min(WG, NW - wg * WG)
        for c in range(N_CHUNKS):
            block_start[(c, wg)] = tb
            tb += wsz * caps[c]
    S_tiles = tb
    S = S_tiles * P

    cores = []
    for k in range(N_CORES):
        sv_all, pd_all = per_core_edges[k]
        w_all = pd_all // P
        slot_all = pd_all % P
        gidx = np.zeros(S, np.int16)
        dcol = np.full(S, PAD_SENTINEL, np.float32)
        for wwin in range(NW):
            m = w_all == wwin
            sv = sv_all[m]
            slots = slot_all[m]
            split = plans[k][wwin]
            # assign each edge to its chunk: reproduce water-fill membership
            used = np.zeros(len(sv), dtype=bool)
            order = np.argsort(sv, kind="stable")
            sv_sorted = sv[order]
            slot_sorted = slots[order]
            wg = wwin // WG
            wi = wwin % WG
            for c in range(N_CHUNKS):
                vals = split[c]          # sorted composite keys for this chunk
                idxs = np.searchsorted(sv_sorted, vals)
                base = block_start[(c, wg)] * P + wi * caps[c] * P
                r = 0
                for v, i0 in zip(vals, idxs):
                    i = i0
                    while used[i] or sv_sorted[i] != v:
                        i += 1
                    used[i] = True
                    pos = base + r
                    # table row: src_core*SPAN + (src_pos - spans[c])
                    gidx[pos] = (v % N_CORES) * SPAN + (v // N_CORES - spans[c])
                    dcol[pos] = slot_sorted[i]
                    r += 1
            assert used.all()
        bpos = batch[k * NPC + node_at_pos[k]]
        bc0 = np.full(NW * P, PAD_SENTINEL, np.float32)
        bc1 = np.full(NW * P, PAD_SENTINEL, np.float32)
        bc0[:NPC] = np.where(bpos < 128, bpos, PAD_SENTINEL)
        bc1[:NPC] = np.where(bpos >= 128, bpos - 128, PAD_SENTINEL)
        # stacked [128 slots, 2, NW] -> stored as [128, 2*NW]
        bcs = np.stack([bc0.reshape(NW, P).T, bc1.reshape(NW, P).T],
                       axis=1).reshape(P, 2 * NW)
        # x in (slot, window) grid layout: one contiguous DMA into h_row/hT
        xk = x[k * NPC:(k + 1) * NPC][node_at_pos[k]].astype(np.float32)
        xg = np.zeros((NW * P, D), np.float32)
        xg[:NPC] = xk
        xs_grid = np.ascontiguousarray(
            xg.reshape(NW, P, D).transpose(1, 0, 2).reshape(P, NW * D))
        xsT_grid = np.ascontiguousarray(xg.T)          # [D, NW*P]
        cores.append(dict(
            gidx=_wrap16(gidx),
            dcol=np.ascontiguousarray(
                dcol.reshape(S // P, P).T.astype(ml_dtypes.bfloat16)),
            bcs=np.ascontiguousarray(bcs),
            perm=node_at_pos[k], xs=xs_grid, xsT=xsT_grid,
        ))
    return dict(N=N, D=D, NPC=NPC, NW=NW, NWG=NWG, spans=spans, caps=caps,
                S=S, S_tiles=S_tiles, block_start=block_start, cores=cores)


def _build_program(meta, n_layers, has_bias):
    import concourse.bacc as bacc
    import concourse.mybir as mybir
    import concourse.tile as tile
    from concourse.library_config import mlp as mlp_lib

    N, D, NPC, NW, NWG, S = (meta[z] for z in
                             ("N", "D", "NPC", "NW", "NWG", "S"))
    caps = meta["caps"]
    spans = meta["spans"]
    block_start = meta["block_start"]
    NT_LAST = NPC - (NW - 1) * P
    f32 = mybir.dt.float32
    bf16 = mybir.dt.bfloat16
    AF = mybir.ActivationFunctionType

    nc = bacc.Bacc("TRN2", target_bir_lowering=False, debug=False,
                   num_swdge_queues=4)

    xs = nc.dram_tensor("xs", [128, NW * D], f32, kind="ExternalInput")
    xsT = nc.dram_tensor("xsT", [D, NW * P], f32, kind="ExternalInput")
    gidx = nc.dram_tensor("gidx", [128, S // 16], mybir.dt.int16, kind="ExternalInput")
    dcol = nc.dram_tensor("dcol", [128, S // P], bf16, kind="ExternalInput")
    bcs = nc.dram_tensor("bcs", [128, 2 * NW], f32, kind="ExternalInput")
    iota8 = nc.dram_tensor("iota8", [128, 8 * P], bf16, kind="ExternalInput")
    iota2f = nc.dram_tensor("iota2f", [128, 2 * P], f32, kind="ExternalInput")
    ident = nc.dram_tensor("ident", [128, P], f32, kind="ExternalInput")
    wmat = nc.dram_tensor("wmat", [D, n_layers * D], f32, kind="ExternalInput")
    wihT = nc.dram_tensor("wihT", [D, 3 * D], f32, kind="ExternalInput")
    whhT = nc.dram_tensor("whhT", [D, 3 * D], f32, kind="ExternalInput")
    if has_bias:
        ones1 = nc.dram_tensor("ones1", [1, P], f32, kind="ExternalInput")
        biasA = nc.dram_tensor("biasA", [1, 3 * D], f32, kind="ExternalInput")
        biasB = nc.dram_tensor("biasB", [1, D], f32, kind="ExternalInput")
    pooled = nc.dram_tensor("pooled", [256, D], f32, kind="ExternalOutput")

    # fire collective c for a layer once window t covers rows < spans[c]+SPAN
    qfire = {}
    for c in range(N_CHUNKS):
        qfire[min(-(-(spans[c] + SPAN) // P), NW) - 1] = c

    with tile.TileContext(nc) as tc:
        with (
            tc.tile_pool(name="const", bufs=1) as cpool,
            tc.tile_pool(name="state", bufs=1) as spool,
            tc.tile_pool(name="gbuf", bufs=20) as gpool,
            tc.tile_pool(name="oh", bufs=12) as opool,
            tc.tile_pool(name="aggw", bufs=6) as apool,
            tc.tile_pool(name="tmp", bufs=12) as tpool,
            tc.tile_pool(name="red", bufs=4, space="PSUM") as rpool,
            tc.tile_pool(name="gruA", bufs=2, space="PSUM") as upoolA,
            tc.tile_pool(name="gruB", bufs=2, space="PSUM") as upoolB,
            tc.tile_pool(name="dram", bufs=1, space="DRAM") as dpool,
        ):
            gidx_sb = spool.tile([128, S // 16], mybir.dt.int16, tag="gidx")
            dcol_sb = spool.tile([128, S // P], bf16, tag="dcol")
            h_row = spool.tile([128, NW, D], f32, tag="hrow")
            hT = spool.tile([64, NW * P], f32, tag="hT")
            iota8_sb = cpool.tile([128, 8 * P], bf16, tag="iota8")
            iota2f_sb = cpool.tile([128, 2 * P], f32, tag="iota2f")
            ident_sb = cpool.tile([128, P], f32, tag="ident")
            pool_acc = spool.tile([128, 2, D], f32, tag="poolacc")
            zro512 = cpool.tile([1, WG * P], bf16, tag="zro512")
            wmat_sb = cpool.tile([D, n_layers * D], f32, tag="wmat")
            wihT_sb = cpool.tile([D, 3 * D], f32, tag="wihT")
            whhT_sb = cpool.tile([D, 3 * D], f32, tag="whhT")
            bcs_sb = cpool.tile([128, 2, NW], f32, tag="bcs")

            nc.sync.dma_start(gidx_sb[:], gidx[:])
            nc.sync.dma_start(dcol_sb[:], dcol[:])
            nc.sync.dma_start(iota8_sb[:], iota8[:])
            nc.sync.dma_start(iota2f_sb[:], iota2f[:])
            nc.sync.dma_start(ident_sb[:], ident[:])
            nc.vector.memset(pool_acc[:], 0.0)
            nc.vector.memset(zro512[:], 0.0)
            nc.sync.dma_start(wmat_sb[:], wmat[:])
            nc.sync.dma_start(wihT_sb[:], wihT[:])
            nc.sync.dma_start(whhT_sb[:], whhT[:])
            nc.sync.dma_start(bcs_sb[:], bcs[:].rearrange("p (a t) -> p a t", a=2))
            if has_bias:
                ones1_sb = cpool.tile([1, P], f32, tag="ones1")
                biasA_sb = cpool.tile([1, 3 * D], f32, tag="biasA")
                biasB_sb = cpool.tile([1, D], f32, tag="biasB")
                nc.sync.dma_start(ones1_sb[:], ones1[:])
                nc.sync.dma_start(biasA_sb[:], biasA[:])
                nc.sync.dma_start(biasB_sb[:], biasB[:])

            m_bounces = []
            m_tables = []
            for layer in range(n_layers):
                mb = dpool.tile([NPC, 2 * D], bf16, tag=f"mb{layer}", name=f"mb{layer}")
                tabs = [dpool.tile([N_CORES * SPAN, 2 * D], bf16,
                                   addr_space="Shared", tag=f"tab{layer}_{c}",
                                   name=f"tab{layer}_{c}")
                        for c in range(N_CHUNKS)]
                m_bounces.append(mb)
                m_tables.append(tabs)

            def emit_m_tile(layer, t, rows, psm):
                nc.tensor.matmul(psm, lhsT=hT[:, t * P:(t + 1) * P],
                                 rhs=wmat_sb[:, layer * D:(layer + 1) * D],
                                 start=True, stop=True)
                mt = tpool.tile([128, 2 * D], bf16, tag="mt", name=f"mt{layer}_{t}")
                nc.scalar.activation(mt[:, 0:D], psm, AF.Copy)
                nc.scalar.activation(mt[:, D:2 * D], psm, AF.Copy)
                nc.sync.dma_start(
                    m_bounces[layer][t * P:t * P + rows, :], mt[:rows, :])

            def emit_collective(layer, c):
                nc.gpsimd.collective_compute(
                    "AllGather", mybir.AluOpType.bypass,
                    ins=[m_bounces[layer][spans[c]:spans[c] + SPAN, :]],
                    outs=[m_tables[layer][c][:]],
                    replica_groups=[list(range(N_CORES))],
                )

            # ---- init: load x -> h_row/hT (pre-transposed), layer-0 m ----
            nc.sync.dma_start(
                h_row[:], xs[:].rearrange("p (t d) -> p t d", d=D))
            nc.sync.dma_start(hT[:], xsT[:])
            for t in range(NW):
                psM = upoolB.tile([128, 320], f32, tag="psM", name=f"psMi{t}")
                emit_m_tile(0, t, P if t < NW - 1 else NT_LAST,
                            psM[:, D:2 * D])
                if t in qfire:
                    emit_collective(0, qfire[t])

            nc.gpsimd.load_library(mlp_lib)

            inst_q = [0]

            def emit_group_gathers(layer, wg):
                """Gather + onehot + reduce matmuls for one window group.

                All wsz windows of the group accumulate into one PSUM bank
                (ps4[:, wi*P:(wi+1)*P]); a single ScalarE copy moves the
                finished group into SBUF.
                """
                wsz = min(WG, NW - wg * WG)
                ps4 = rpool.tile([64, WG * P], f32, tag="red",
                                 name=f"ps4_{layer}_{wg}")
                aggw4 = apool.tile([64, WG * P], f32, tag="aggw",
                                   name=f"aggw{layer}_{wg}")
                # bank opener: one start=True covering the whole zero region,
                # so every later slice-matmul (start=False) first-touch zeroes
                nc.tensor.matmul(ps4[:], lhsT=zro512[:, 0:D], rhs=zro512[:],
                                 start=True, stop=False)
                last_mm = (N_CHUNKS - 1, wsz * caps[N_CHUNKS - 1] - 1)
                for c in range(N_CHUNKS):
                    g0 = block_start[(c, wg)]
                    rtiles = wsz * caps[c]
                    tab_win = m_tables[layer][c][:]
                    j = 0
                    while j < rtiles:
                        tiles_this = min(8, rtiles - j)
                        ni = tiles_this * P
                        gb = gpool.tile([128, 8, 2 * D], bf16, tag="gb",
                                        name=f"gb{layer}_{wg}_{c}_{j}")
                        off = (g0 + j) * 8
                        nc.gpsimd.dma_gather(
                            gb[:, :tiles_this, :], tab_win,
                            gidx_sb[:, off:off + ni // 16],
                            ni, ni, 2 * D, queue_num=inst_q[0] % 4,
                        )
                        inst_q[0] += 1
                        gt = g0 + j
                        oht = opool.tile([128, 8, P], bf16, tag="oh",
                                         name=f"oh{layer}_{wg}_{c}_{j}")
                        nc.vector.tensor_tensor(
                            out=oht[:, :tiles_this, :],
                            in0=dcol_sb[:, gt:gt + tiles_this].to_broadcast(
                                [128, tiles_this, P]),
                            in1=iota8_sb[:].rearrange(
                                "p (a b) -> p a b", a=8)[:, :tiles_this, :],
                            op=mybir.AluOpType.is_equal,
                        )
                        for u in range(tiles_this):
                            tt = j + u
                            wi = tt // caps[c]
                            nc.tensor.matmul(
                                ps4[:, wi * P:(wi + 1) * P],
                                lhsT=gb[:, u, 0:D], rhs=oht[:, u, :],
                                start=False, stop=(c, tt) == last_mm,
                            )
                        j += tiles_this
                nc.scalar.activation(aggw4[:, 0:wsz * P], ps4[:, 0:wsz * P],
                                     AF.Copy)
                return aggw4

            def emit_gru_tile(layer, t, aggw):
                # psA gets its own bank (it accumulates); psB/psm/pst (single
                # write each) share one bank — order-independent via the
                # pending-zero first-touch semantics.
                sl = slice(t * P, (t + 1) * P)
                psA = upoolA.tile([128, 3 * D], f32, tag="psA",
                                  name=f"psA{layer}_{t}")
                psM = upoolB.tile([128, 320], f32, tag="psM",
                                  name=f"psM{layer}_{t}")
                psB = psM[:, 0:D]
                nc.tensor.matmul(psA[:, 0:2 * D], lhsT=hT[:, sl],
                                 rhs=whhT_sb[:, 0:2 * D], start=True, stop=False)
                nc.tensor.matmul(psA[:], lhsT=aggw, rhs=wihT_sb[:],
                                 start=False, stop=not has_bias)
                if has_bias:
                    nc.tensor.matmul(psA[:], lhsT=ones1_sb[:], rhs=biasA_sb[:],
                                     start=False, stop=True)
                nc.tensor.matmul(psB, lhsT=hT[:, sl],
                                 rhs=whhT_sb[:, 2 * D:3 * D], start=True,
                                 stop=not has_bias)
                if has_bias:
                    nc.tensor.matmul(psB, lhsT=ones1_sb[:], rhs=biasB_sb[:],
                                     start=False, stop=True)
                rz = tpool.tile([128, 2 * D], f32, tag="rz", name=f"rz{layer}_{t}")
                nn = tpool.tile([128, D], f32, tag="nn", name=f"nn{layer}_{t}")
                t1 = tpool.tile([128, D], f32, tag="t1", name=f"t1{layer}_{t}")
                nc.scalar.activation(rz[:], psA[:, 0:2 * D], AF.Sigmoid)
                nc.vector.tensor_tensor(out=t1[:], in0=rz[:, 0:D], in1=psB,
                                        op=mybir.AluOpType.mult)
                nc.vector.tensor_tensor(out=t1[:], in0=t1[:],
                                        in1=psA[:, 2 * D:3 * D],
                                        op=mybir.AluOpType.add)
                nc.scalar.activation(nn[:], t1[:], AF.Tanh)
                nc.vector.tensor_tensor(out=t1[:], in0=h_row[:, t, :], in1=nn[:],
                                        op=mybir.AluOpType.subtract)
                nc.vector.tensor_tensor(out=t1[:], in0=rz[:, D:2 * D], in1=t1[:],
                                        op=mybir.AluOpType.mult)
                nc.vector.tensor_tensor(out=h_row[:, t, :], in0=nn[:], in1=t1[:],
                                        op=mybir.AluOpType.add)
                if layer < n_layers - 1:
                    pst = psM[0:64, 2 * D:4 * D]
                    nc.tensor.transpose(pst, h_row[:, t, :], ident_sb[:])
                    nc.scalar.activation(hT[:, sl], pst, AF.Copy)
                    emit_m_tile(layer + 1, t, P if t < NW - 1 else NT_LAST,
                                psM[:, D:2 * D])
                else:
                    # fold graph pooling into the last wave: per-window
                    # one-hot matmuls + one DVE accumulate into pool_acc
                    oh0 = opool.tile([128, 2, P], f32, tag="oh",
                                     name=f"ohp{t}")
                    nc.vector.tensor_tensor(
                        out=oh0[:],
                        in0=bcs_sb[:, :, t].to_broadcast([128, 2, P]),
                        in1=iota2f_sb[:].rearrange("p (a b) -> p a b", a=2),
                        op=mybir.AluOpType.is_equal)
                    nc.tensor.matmul(psM[:, D:2 * D], lhsT=oh0[:, 0, :],
                                     rhs=h_row[:, t, :], start=True, stop=True)
                    nc.tensor.matmul(psM[:, 2 * D:3 * D], lhsT=oh0[:, 1, :],
                                     rhs=h_row[:, t, :], start=True, stop=True)
                    nc.vector.tensor_tensor(
                        out=pool_acc[:],
                        in0=pool_acc[:],
                        in1=psM[:, D:3 * D].rearrange(
                            "p (a b) -> p a b", a=2),
                        op=mybir.AluOpType.add)

            for layer in range(n_layers):
                for wg in range(NWG):
                    aggw4 = emit_group_gathers(layer, wg)
                    wsz = min(WG, NW - wg * WG)
                    for wi in range(wsz):
                        t = wg * WG + wi
                        emit_gru_tile(layer, t, aggw4[:, wi * P:(wi + 1) * P])
                        if layer < n_layers - 1 and t in qfire:
                            emit_collective(layer + 1, qfire[t])

            # ---- pooling output (accumulated during the last wave) ----
            nc.sync.dma_start(pooled[0:128, :], pool_acc[:, 0, :])
            nc.sync.dma_start(pooled[128:256, :], pool_acc[:, 1, :])

    nc.compile()
    return nc


def kernel(x, edge_index, batch, weight, W_ih, W_hh, b_ih, b_hh,
           _trace=False):
    from concourse.bass_utils import run_bass_kernel_spmd

    x = np.asarray(x, np.float32)
    weight = np.asarray(weight, np.float32)
    W_ih = np.asarray(W_ih, np.float32)
    W_hh = np.asarray(W_hh, np.float32)
    b_ih = np.asarray(b_ih, np.float32)
    b_hh = np.asarray(b_hh, np.float32)
    N, D = x.shape
    n_layers = weight.shape[0]
    NPC = N // N_CORES

    meta = _host_prep(x, edge_index, batch)
    has_bias = bool(np.any(b_ih) or np.any(b_hh))
    key = (N, D, n_layers, meta["spans"], meta["caps"], has_bias)
    if key not in _cache:
        _cache[key] = _build_program(meta, n_layers, has_bias)
    nc = _cache[key]

    iota8_np = np.tile(np.arange(P, dtype=np.float32),
                       (128, 8)).astype(ml_dtypes.bfloat16)
    iota2f_np = np.tile(np.arange(P, dtype=np.float32), (128, 2))
    ident_np = np.eye(P, dtype=np.float32)
    wmat_np = np.concatenate([weight[i] for i in range(n_layers)], axis=1)
    wihT_np = np.ascontiguousarray(W_ih.T)
    whhT_np = np.ascontiguousarray(W_hh.T)

    in_maps = []
    for k in range(N_CORES):
        ck = meta["cores"][k]
        im = dict(
            xs=ck["xs"], xsT=ck["xsT"],
            gidx=ck["gidx"], dcol=ck["dcol"], bcs=ck["bcs"],
            iota8=iota8_np, iota2f=iota2f_np, ident=ident_np,
            wmat=wmat_np, wihT=wihT_np, whhT=whhT_np,
        )
        if has_bias:
            im["ones1"] = np.ones((1, P), np.float32)
            im["biasA"] = np.concatenate([
                b_ih[0:D] + b_hh[0:D], b_ih[D:2 * D] + b_hh[D:2 * D],
                b_ih[2 * D:3 * D]]).reshape(1, 3 * D).astype(np.float32)
            im["biasB"] = b_hh[2 * D:3 * D].reshape(1, D).astype(np.float32)
        in_maps.append(im)

    res = run_bass_kernel_spmd(nc, in_maps, core_ids=list(range(N_CORES)),
                               trace=_trace)
    out = np.zeros((256, D), np.float32)
    for k in range(N_CORES):
        out += res.results[k]["pooled"]
    kernel._last_exec_time_ns = res.exec_time_ns
    kernel._last_result = res
    return out
